# revision 18
# baseline (speedup 1.0000x reference)
"""Batched GAT kernel for 8 Trainium2 NeuronCores.

Math: out[b,i,:] = softmax_j(mask(leakyrelu(s_i+t_j))) @ h  per head, concat heads.

Decomposition: exp(lrelu(e)) = max(u_i v_j, u'_i v'_j) with u=exp(s), v=exp(t),
u'=exp(.2s), v'=exp(.2t).  Dividing each row i by u'_i (cancels in softmax):
  p~_ij = m_ij * max(w_i v_j, v'_j),   w = exp(.8 s)
  out = (P~ @ [h|1]) -> num/den per head.  No G-indicator, no mask matmuls,
  no u-rescale combine: one matmul stream against a plain [h|1] fp16 pack.

Per core (c = 0..7): b = c//2, rows i in [ (c%2)*1024, +1024 ).
Per (head, j-tile): q = tensor_scalar(wb, *v_j, max v'_j) (DVE 4x) and
p~ = min(q, maskT) (DVE/GPSIMD 2x) where maskT in {0, BIG} comes from the
binarized adj transposed on PE via a regular matmul against BIG*I (scales the
mask for free).  Finalize per head: reciprocal_approx_fast on the den row,
fp16 ones-broadcast matmul, scale+bias, PE transpose out.
"""
import os
import sys
import numpy as np

for _p in ("/opt/trn_rl_repo",):
    if _p not in sys.path:
        sys.path.insert(0, _p)

B, N, D, H, F = 4, 2048, 128, 4, 32
HF = H * F           # 128
IR = 1024            # i-rows per core
NJT = N // 128       # 16 j-tiles
NCORES = 8

# engine assignment knobs
ACT_BIN = set(range(8))            # binarize blocks on Act (sigmoid)
POOL_BIN = set()                   # binarize blocks on GPSIMD (is_gt)


def _on_pool(h, jt):
    # mask-mult engine choice: GPSIMD for a balanced subset, DVE otherwise
    if h == 3:
        return jt % 2 == 1
    if h == 2 and jt >= 14:
        return False
    return (jt + 5 * h) % 3 == 0

_CACHE = {}


def build_nc(reps=1):
    import concourse.bacc as bacc
    import concourse.tile as tile
    from concourse import mybir

    f32, f16 = mybir.dt.float32, mybir.dt.float16
    Alu = mybir.AluOpType
    Act = mybir.ActivationFunctionType

    nc = bacc.Bacc(None, target_bir_lowering=False)

    xT_d   = nc.dram_tensor("xT",   [D, N],    f32, kind="ExternalInput")
    xiT_d  = nc.dram_tensor("xiT",  [D, IR],   f32, kind="ExternalInput")
    adj_d  = nc.dram_tensor("adjS", [IR, N],   f32, kind="ExternalInput")
    Wf_d   = nc.dram_tensor("Wf",   [D, HF],   f32, kind="ExternalInput")
    aS_d   = nc.dram_tensor("aS",   [HF, H],   f32, kind="ExternalInput")
    aD_d   = nc.dram_tensor("aD",   [HF, H],   f32, kind="ExternalInput")
    bias_d = nc.dram_tensor("biasR", [1, HF],  f32, kind="ExternalInput")
    out_d  = nc.dram_tensor("out",  [IR, HF],  f32, kind="ExternalOutput")

    # host constants
    EY = np.zeros((4, 4 * 128), np.float16)
    for h in range(H):
        EY[h, h * 128:(h + 1) * 128] = 1.0
    EY_d = nc.inline_tensor(EY, "EYc")
    ID16_d = nc.inline_tensor(np.eye(128, dtype=np.float16), "id16c")


    adj_r = adj_d[:].rearrange("(s p) j -> p s j", p=128)

    with tile.TileContext(nc) as tc:
        cst_ctx = tc.tile_pool(name="cst", bufs=1)
        cst = cst_ctx.__enter__()
        try:
            xT   = cst.tile([D, N], f32)
            xiT  = cst.tile([D, IR], f32)
            Wf   = cst.tile([D, HF], f32)
            aS   = cst.tile([HF, H], f32)
            aD   = cst.tile([HF, H], f32)
            biasR = cst.tile([1, HF], f32)
            biasTE = cst.tile([64, 4, 33], f16)
            eyc  = cst.tile([4, 4 * 128], f16)
            id16c = cst.tile([128, 128], f16)
            sigB = cst.tile([128, 1], f32)

            Wf16 = cst.tile([D, HF], f16)
            aS16 = cst.tile([HF, H], f16)
            aD16 = cst.tile([HF, H], f16)
            xT16 = cst.tile([D, N], f16)
            xiT16 = cst.tile([D, IR], f16)
            hT16 = cst.tile([HF, N], f16)
            hiT16 = cst.tile([HF, IR], f16)
            warmA = cst.tile([1, 4], f32)
            tAll = cst.tile([128, NJT, H], f32)
            tv1  = cst.tile([128, NJT, H], f32)   # exp(t)
            tv2  = cst.tile([128, NJT, H], f32)   # exp(.2 t)
            sZ4  = cst.tile([4, IR], f32)
            w16  = cst.tile([4, IR], f16)         # exp(.8 s) fp16
            wb16 = cst.tile([128, H, IR], f16)    # broadcast of w16 per head
            Vpack = cst.tile([128, NJT, H, 33], f16)
            mT_all = cst.tile([128, NJT, IR], f16)
            out_sb = cst.tile([128, 8, HF], f32)

            nc.sync.dma_start(Wf[:], Wf_d[:])
            nc.sync.dma_start(xiT[:], xiT_d[:])
            nc.sync.dma_start(xT[:], xT_d[:])
            nc.sync.dma_start(aS[:], aS_d[:])
            nc.sync.dma_start(aD[:], aD_d[:])
            nc.sync.dma_start(eyc[:], EY_d[:])
            nc.sync.dma_start(id16c[:], ID16_d[:])
            nc.sync.dma_start(biasR[:], bias_d[:])
            nc.vector.memset(sigB[:], -5e5)
            nc.vector.memset(biasTE[:], 0.0)
            nc.scalar.copy(
                biasTE[32:33, :, 0:32],
                biasR[:].rearrange("p (h f) -> p h f", h=H))

            # ---------------- prep ----------------
            # warm every activation-table set during the DMA wait
            nc.scalar.copy(warmA[:, 0:1], sigB[0:1, 0:1])
            nc.scalar.activation(warmA[:, 1:2], sigB[0:1, 0:1], Act.Exp)
            nc.scalar.activation(warmA[:, 2:3], sigB[0:1, 0:1], Act.Sigmoid,
                                 bias=sigB[0:1, 0:1], scale=0.0)
            nc.scalar.copy(Wf16[:], Wf[:])
            nc.scalar.copy(aS16[:], aS[:])
            nc.scalar.copy(aD16[:], aD[:])
            nc.scalar.copy(xiT16[:], xiT[:])
            nc.scalar.copy(xT16[:], xT[:])
            with tc.tile_pool(name="pp", bufs=3, space="PSUM") as pp:
                # s chain: hiT -> sZ4 -> w16 -> wb16 (feeds the TS q-ops)
                for k in range(2):
                    ps = pp.tile([HF, 512], f32, tag="pp")
                    nc.tensor.matmul(ps[:], Wf16[:], xiT16[:, k * 512:(k + 1) * 512],
                                     start=True, stop=True)
                    nc.vector.tensor_copy(hiT16[:, k * 512:(k + 1) * 512], ps[:])
                for k in range(2):
                    ps = pp.tile([4, 512], f32, tag="pp")
                    nc.tensor.matmul(ps[:], aS16[:], hiT16[:, k * 512:(k + 1) * 512],
                                     start=True, stop=True)
                    nc.scalar.copy(sZ4[:, k * 512:(k + 1) * 512], ps[:])
                nc.scalar.activation(w16[:], sZ4[:], Act.Exp, scale=0.8)
                # t chain: hT -> tAll -> exps (feeds the TS scalars)
                for k in range(4):
                    ps = pp.tile([HF, 512], f32, tag="pp")
                    nc.tensor.matmul(ps[:], Wf16[:], xT16[:, k * 512:(k + 1) * 512],
                                     start=True, stop=True)
                    nc.vector.tensor_copy(hT16[:, k * 512:(k + 1) * 512], ps[:])
                for g in range(4):
                    ps = pp.tile([128, 4 * H], f32, tag="pp")
                    for k4 in range(4):
                        jt = g * 4 + k4
                        nc.tensor.matmul(ps[:, k4 * H:(k4 + 1) * H],
                                         hT16[:, jt * 128:(jt + 1) * 128], aD16[:],
                                         start=True, stop=True)
                    nc.scalar.copy(tAll[:, g * 4:(g + 1) * 4, :], ps[:])
                nc.scalar.activation(
                    tv1[:].rearrange("p a b -> p (a b)"),
                    tAll[:].rearrange("p a b -> p (a b)"), Act.Exp)
                nc.scalar.activation(
                    tv2[:].rearrange("p a b -> p (a b)"),
                    tAll[:].rearrange("p a b -> p (a b)"), Act.Exp, scale=0.2)
                # wb16: broadcast w16 rows to 128 partitions via PE
                for h in range(H):
                    for k in range(2):
                        ps = pp.tile([128, 512], f32, tag="pp")
                        nc.tensor.matmul(ps[:], eyc[:, h * 128:(h + 1) * 128],
                                         w16[:, k * 512:(k + 1) * 512],
                                         start=True, stop=True)
                        nc.vector.tensor_copy(wb16[:, h, k * 512:(k + 1) * 512], ps[:])
                # Vpack ones column (h/16 cols are drained inside the blk loop)
                nc.gpsimd.memset(Vpack[:, :, :, 32:33], 0.0625)

            # ---------------- main body (per rep) ----------------
            def pair_ops(gqp, pgt, h, jt):
                q = gqp.tile([128, IR], f16, tag="q")
                nc.vector.tensor_scalar(q[:], wb16[:, h, :],
                                        tv1[:, jt, h:h + 1],
                                        tv2[:, jt, h:h + 1],
                                        op0=Alu.mult, op1=Alu.max)
                pt = gqp.tile([128, IR], f16, tag="pt")
                eng = nc.gpsimd if _on_pool(h, jt) else nc.vector
                eng.tensor_tensor(pt[:], q[:], mT_all[:, jt, :], op=Alu.mult)
                tile, off = pgt[h]
                for k in range(2):
                    nc.tensor.matmul(tile[off:off + 33, k * 512:(k + 1) * 512],
                                     Vpack[:, jt, h, :],
                                     pt[:, k * 512:(k + 1) * 512],
                                     start=(jt == 0), stop=(jt == NJT - 1))

            def fin(ftp, ndp, pgt, h):
                tile, off = pgt[h]
                numD = ndp.tile([33, IR], f16, tag="numD")
                nc.scalar.copy(numD[:], tile[off:off + 33, :])
                tpA = ftp.tile([128, 8, 33], f32, tag="tpA")
                for c in range(8):
                    nc.tensor.matmul(tpA[:, c, :],
                                     numD[:, c * 128:(c + 1) * 128],
                                     id16c[0:33, 0:33], start=True, stop=False)
                    nc.tensor.matmul(tpA[:, c, :],
                                     numD[32:33, c * 128:(c + 1) * 128],
                                     biasTE[32:33, h, :], start=False, stop=True)
                rdT = ndp.tile([128, 8, 1], f32, tag="rdT")
                nc.vector.reciprocal_approx_fast(rdT[:], tpA[:, :, 32:33])
                nc.vector.tensor_tensor(
                    out_sb[:, :, h * 32:(h + 1) * 32], tpA[:, :, 0:32],
                    rdT[:, :, 0:1].broadcast_to([128, 8, 32]), op=Alu.mult)
                nc.sync.dma_start(
                    out_d[:, h * 32:(h + 1) * 32].rearrange(
                        "(s p) f -> p s f", p=128),
                    out_sb[:, :, h * 32:(h + 1) * 32])

            def emit_body():
                psg_ctx = tc.tile_pool(name="psg", bufs=2, space="PSUM")
                psg = psg_ctx.__enter__()
                gqp_ctx = tc.tile_pool(name="gqp", bufs=10)
                gqp = gqp_ctx.__enter__()
                pgA = psg.tile([97, IR], f32, tag="pg", name="pgA")
                pgB = psg.tile([97, IR], f32, tag="pg", name="pgB")
                pgt = {0: (pgA, 0), 1: (pgA, 64), 2: (pgB, 0), 3: (pgB, 64)}

                # phase 1: masks stream; h0/h1 consume at production rate,
                # h2 lags two j-tiles
                with tc.tile_pool(name="adjp", bufs=2) as adjp, \
                     tc.tile_pool(name="mip", bufs=2) as mip, \
                     tc.tile_pool(name="mtp", bufs=2, space="PSUM") as mtp, \
                     tc.tile_pool(name="pvp", bufs=1, space="PSUM") as pvp:
                    for blk in range(8):
                        at = adjp.tile([128, 8, 256], f32, tag="adj")
                        nc.sync.dma_start(at[:], adj_r[:, :, blk * 256:(blk + 1) * 256])
                        mi = mip.tile([128, 8, 256], f16, tag="mi")
                        if blk in ACT_BIN:
                            nc.scalar.activation(
                                mi[:].rearrange("p a b -> p (a b)"),
                                at[:].rearrange("p a b -> p (a b)"),
                                Act.Sigmoid, bias=sigB[:, 0:1], scale=1e6)
                        else:
                            nc.vector.tensor_scalar(mi[:], at[:], 0.5, None,
                                                    op0=Alu.is_gt)
                        jt0 = 2 * blk
                        pv = pvp.tile([128, 512], f32, tag="pv", name="pv")
                        for d in range(2):
                            nc.tensor.matmul(
                                pv[:, d * 128:(d + 1) * 128],
                                xT16[:, (jt0 + d) * 128:(jt0 + d + 1) * 128],
                                Wf16[:], start=True, stop=True)
                        nc.vector.tensor_scalar(
                            Vpack[:, jt0:jt0 + 2, :, 0:32],
                            pv[:, 0:256].rearrange("p (j h f) -> p j h f", j=2, h=H),
                            0.0625, None, op0=Alu.mult)
                        for q in range(2):
                            jt = jt0 + q
                            mt = mtp.tile([128, IR], f16, tag="mt16", name="mt16")
                            for s in range(8):
                                nc.tensor.transpose(
                                    mt[:, s * 128:(s + 1) * 128],
                                    mi[:, s, q * 128:(q + 1) * 128], id16c[:])
                            nc.scalar.copy(mT_all[:, jt, :], mt[:])
                            pair_ops(gqp, pgt, 0, jt)
                            pair_ops(gqp, pgt, 1, jt)
                            if jt >= 2:
                                pair_ops(gqp, pgt, 2, jt - 2)

                # phase 2: h2 tail, h3, pipelined fins
                ftp_ctx = tc.tile_pool(name="ftp", bufs=2, space="PSUM")
                ftp = ftp_ctx.__enter__()
                ndp_ctx = tc.tile_pool(name="ndp", bufs=4)
                ndp = ndp_ctx.__enter__()
                try:
                    pair_ops(gqp, pgt, 2, NJT - 2)
                    pair_ops(gqp, pgt, 2, NJT - 1)
                    fin(ftp, ndp, pgt, 0)
                    for jt in range(NJT):
                        pair_ops(gqp, pgt, 3, jt)
                        if jt == 2:
                            fin(ftp, ndp, pgt, 1)
                        elif jt == 6:
                            fin(ftp, ndp, pgt, 2)
                    fin(ftp, ndp, pgt, 3)
                finally:
                    ndp_ctx.__exit__(None, None, None)
                    ftp_ctx.__exit__(None, None, None)
                    gqp_ctx.__exit__(None, None, None)
                    psg_ctx.__exit__(None, None, None)

            for _rep in range(reps):
                emit_body()
        finally:
            cst_ctx.__exit__(None, None, None)

    nc.compile()
    return nc


def _prepare_in_maps(x, adj, W, a_src, a_dst, bias):
    x = np.ascontiguousarray(np.asarray(x, dtype=np.float32))
    adj = np.asarray(adj, dtype=np.float32)
    W = np.asarray(W, dtype=np.float32)
    a_src = np.asarray(a_src, dtype=np.float32)
    a_dst = np.asarray(a_dst, dtype=np.float32)
    bias = np.asarray(bias, dtype=np.float32)

    Wf = np.ascontiguousarray(W.reshape(D, HF))
    aS = np.zeros((HF, H), np.float32)
    aD = np.zeros((HF, H), np.float32)
    for h in range(H):
        aS[h * F:(h + 1) * F, h] = a_src[h]
        aD[h * F:(h + 1) * F, h] = a_dst[h]
    biasRh = np.ascontiguousarray(bias.reshape(1, HF))

    in_maps = []
    for c in range(NCORES):
        b, cc = c // 2, c % 2
        i0 = cc * IR
        in_maps.append({
            "xT": np.ascontiguousarray(x[b].T),
            "xiT": np.ascontiguousarray(x[b, i0:i0 + IR].T),
            "adjS": np.ascontiguousarray(adj[b, i0:i0 + IR, :]),
            "Wf": Wf,
            "aS": aS,
            "aD": aD,
            "biasR": biasRh,
        })
    return in_maps


def run(inputs, trace=False, trace_cores=None):
    from concourse.bass_utils import run_bass_kernel_spmd
    if "nc" not in _CACHE:
        _CACHE["nc"] = build_nc()
    nc = _CACHE["nc"]
    in_maps = _prepare_in_maps(**inputs)
    kw = {}
    if trace:
        kw = dict(trace=True, trace_cores=trace_cores or [0])
    res = run_bass_kernel_spmd(nc, in_maps, list(range(NCORES)), **kw)
    out = np.zeros((B, N, HF), np.float32)
    for c in range(NCORES):
        b, cc = c // 2, c % 2
        out[b, cc * IR:(cc + 1) * IR, :] = res.results[c]["out"]
    return out, res


def kernel(**inputs):
    out, _ = run(inputs, trace=False)
    return out


# revision 19
# speedup vs baseline: 1.1246x; 1.1246x over previous
"""Batched GAT kernel for 8 Trainium2 NeuronCores.

Math: out[b,i,:] = softmax_j(mask(leakyrelu(s_i+t_j))) @ h  per head, concat heads.

Decomposition: exp(lrelu(e)) = max(u_i v_j, u'_i v'_j) with u=exp(s), v=exp(t),
u'=exp(.2s), v'=exp(.2t).  Dividing each row i by u'_i (cancels in softmax):
  p~_ij = m_ij * max(w_i v_j, v'_j),   w = exp(.8 s)
  out = (P~ @ [h|1]) -> num/den per head.  No G-indicator, no mask matmuls,
  no u-rescale combine: one matmul stream against a plain [h|1] fp16 pack.

Per core (c = 0..7): b = c//2, rows i in [ (c%2)*1024, +1024 ).
Per (head, j-tile): q = tensor_scalar(wb, *v_j, max v'_j) (DVE 4x) and
p~ = min(q, maskT) (DVE/GPSIMD 2x) where maskT in {0, BIG} comes from the
binarized adj transposed on PE via a regular matmul against BIG*I (scales the
mask for free).  Finalize per head: reciprocal_approx_fast on the den row,
fp16 ones-broadcast matmul, scale+bias, PE transpose out.
"""
import os
import sys
import numpy as np

for _p in ("/opt/trn_rl_repo",):
    if _p not in sys.path:
        sys.path.insert(0, _p)

B, N, D, H, F = 4, 2048, 128, 4, 32
HF = H * F           # 128
IR = 1024            # i-rows per core
NJT = N // 128       # 16 j-tiles
NCORES = 8

# engine assignment knobs
ACT_BIN = set(range(8))            # binarize blocks on Act (sigmoid)
POOL_BIN = set()                   # binarize blocks on GPSIMD (is_gt)


def _on_pool(h, jt):
    # mask-mult engine choice: the lagged head (h2) runs on GPSIMD in phase 1
    # (its 2-jt lag absorbs the slower engine), h3 alternates in phase 2
    if h == 2:
        return jt < 14
    if h == 3:
        return jt % 2 == 1
    return False

_CACHE = {}


def build_nc(reps=1):
    import concourse.bacc as bacc
    import concourse.tile as tile
    from concourse import mybir

    f32, f16 = mybir.dt.float32, mybir.dt.float16
    Alu = mybir.AluOpType
    Act = mybir.ActivationFunctionType

    nc = bacc.Bacc(None, target_bir_lowering=False)

    xT_d   = nc.dram_tensor("xT",   [D, N],    f32, kind="ExternalInput")
    xiT_d  = nc.dram_tensor("xiT",  [D, IR],   f32, kind="ExternalInput")
    adj_d  = nc.dram_tensor("adjS", [IR, N],   f32, kind="ExternalInput")
    Wf_d   = nc.dram_tensor("Wf",   [D, HF],   f32, kind="ExternalInput")
    aS_d   = nc.dram_tensor("aS",   [HF, H],   f32, kind="ExternalInput")
    aD_d   = nc.dram_tensor("aD",   [HF, H],   f32, kind="ExternalInput")
    bias_d = nc.dram_tensor("biasR", [1, HF],  f32, kind="ExternalInput")
    out_d  = nc.dram_tensor("out",  [IR, HF],  f32, kind="ExternalOutput")

    # host constants
    EY = np.zeros((4, 4 * 128), np.float16)
    for h in range(H):
        EY[h, h * 128:(h + 1) * 128] = 1.0
    EY_d = nc.inline_tensor(EY, "EYc")
    ID16_d = nc.inline_tensor(np.eye(128, dtype=np.float16), "id16c")


    adj_r = adj_d[:].rearrange("(s p) j -> p s j", p=128)

    with tile.TileContext(nc) as tc:
        cst_ctx = tc.tile_pool(name="cst", bufs=1)
        cst = cst_ctx.__enter__()
        try:
            xT   = cst.tile([D, N], f32)
            xiT  = cst.tile([D, IR], f32)
            Wf   = cst.tile([D, HF], f32)
            aS   = cst.tile([HF, H], f32)
            aD   = cst.tile([HF, H], f32)
            biasR = cst.tile([1, HF], f32)
            biasTE = cst.tile([64, 4, 33], f16)
            eyc  = cst.tile([4, 4 * 128], f16)
            id16c = cst.tile([128, 128], f16)
            sigB = cst.tile([128, 1], f32)

            Wf16 = cst.tile([D, HF], f16)
            aS16 = cst.tile([HF, H], f16)
            aD16 = cst.tile([HF, H], f16)
            xT16 = cst.tile([D, N], f16)
            xiT16 = cst.tile([D, IR], f16)
            hT16 = cst.tile([HF, N], f16)
            hiT16 = cst.tile([HF, IR], f16)
            warmA = cst.tile([1, 4], f32)
            tAll = cst.tile([128, NJT, H], f32)
            tv1  = cst.tile([128, NJT, H], f32)   # exp(t)
            tv2  = cst.tile([128, NJT, H], f32)   # exp(.2 t)
            sZ4  = cst.tile([4, IR], f32)
            w16  = cst.tile([4, IR], f16)         # exp(.8 s) fp16
            wb16 = cst.tile([128, H, IR], f16)    # broadcast of w16 per head
            Vpack = cst.tile([128, NJT, H, 33], f16)
            mT_all = cst.tile([128, NJT, IR], f16)
            out_sb = cst.tile([128, 8, HF], f32)

            nc.sync.dma_start(Wf[:], Wf_d[:])
            nc.sync.dma_start(xiT[:], xiT_d[:])
            nc.sync.dma_start(xT[:], xT_d[:])
            nc.sync.dma_start(aS[:], aS_d[:])
            nc.sync.dma_start(aD[:], aD_d[:])
            nc.sync.dma_start(eyc[:], EY_d[:])
            nc.sync.dma_start(id16c[:], ID16_d[:])
            nc.sync.dma_start(biasR[:], bias_d[:])
            nc.vector.memset(sigB[:], -5e5)
            nc.vector.memset(biasTE[:], 0.0)
            nc.scalar.copy(
                biasTE[32:33, :, 0:32],
                biasR[:].rearrange("p (h f) -> p h f", h=H))

            # ---------------- prep ----------------
            # warm every activation-table set during the DMA wait
            nc.scalar.copy(warmA[:, 0:1], sigB[0:1, 0:1])
            nc.scalar.activation(warmA[:, 1:2], sigB[0:1, 0:1], Act.Exp)
            nc.scalar.activation(warmA[:, 2:3], sigB[0:1, 0:1], Act.Sigmoid,
                                 bias=sigB[0:1, 0:1], scale=0.0)
            nc.scalar.copy(Wf16[:], Wf[:])
            nc.scalar.copy(aS16[:], aS[:])
            nc.scalar.copy(aD16[:], aD[:])
            nc.scalar.copy(xiT16[:], xiT[:])
            nc.scalar.copy(xT16[:], xT[:])
            with tc.tile_pool(name="pp", bufs=3, space="PSUM") as pp:
                # s chain: hiT -> sZ4 -> w16 -> wb16 (feeds the TS q-ops)
                for k in range(2):
                    ps = pp.tile([HF, 512], f32, tag="pp")
                    nc.tensor.matmul(ps[:], Wf16[:], xiT16[:, k * 512:(k + 1) * 512],
                                     start=True, stop=True)
                    nc.vector.tensor_copy(hiT16[:, k * 512:(k + 1) * 512], ps[:])
                for k in range(2):
                    ps = pp.tile([4, 512], f32, tag="pp")
                    nc.tensor.matmul(ps[:], aS16[:], hiT16[:, k * 512:(k + 1) * 512],
                                     start=True, stop=True)
                    nc.scalar.copy(sZ4[:, k * 512:(k + 1) * 512], ps[:])
                nc.scalar.activation(w16[:], sZ4[:], Act.Exp, scale=0.8)
                # t chain: hT -> tAll -> exps (feeds the TS scalars)
                for k in range(4):
                    ps = pp.tile([HF, 512], f32, tag="pp")
                    nc.tensor.matmul(ps[:], Wf16[:], xT16[:, k * 512:(k + 1) * 512],
                                     start=True, stop=True)
                    nc.vector.tensor_copy(hT16[:, k * 512:(k + 1) * 512], ps[:])
                for g in range(4):
                    ps = pp.tile([128, 4 * H], f32, tag="pp")
                    for k4 in range(4):
                        jt = g * 4 + k4
                        nc.tensor.matmul(ps[:, k4 * H:(k4 + 1) * H],
                                         hT16[:, jt * 128:(jt + 1) * 128], aD16[:],
                                         start=True, stop=True)
                    nc.scalar.copy(tAll[:, g * 4:(g + 1) * 4, :], ps[:])
                nc.scalar.activation(
                    tv1[:].rearrange("p a b -> p (a b)"),
                    tAll[:].rearrange("p a b -> p (a b)"), Act.Exp)
                nc.scalar.activation(
                    tv2[:].rearrange("p a b -> p (a b)"),
                    tAll[:].rearrange("p a b -> p (a b)"), Act.Exp, scale=0.2)
                # wb16: broadcast w16 rows to 128 partitions via PE
                for h in range(H):
                    for k in range(2):
                        ps = pp.tile([128, 512], f32, tag="pp")
                        nc.tensor.matmul(ps[:], eyc[:, h * 128:(h + 1) * 128],
                                         w16[:, k * 512:(k + 1) * 512],
                                         start=True, stop=True)
                        nc.vector.tensor_copy(wb16[:, h, k * 512:(k + 1) * 512], ps[:])
                # Vpack ones column (h/16 cols are drained inside the blk loop)
                nc.gpsimd.memset(Vpack[:, :, :, 32:33], 0.0625)

            # ---------------- main body (per rep) ----------------
            def pair_ops(gqp, pgt, h, jt):
                q = gqp.tile([128, IR], f16, tag="q")
                nc.vector.tensor_scalar(q[:], wb16[:, h, :],
                                        tv1[:, jt, h:h + 1],
                                        tv2[:, jt, h:h + 1],
                                        op0=Alu.mult, op1=Alu.max)
                pt = gqp.tile([128, IR], f16, tag="pt")
                eng = nc.gpsimd if _on_pool(h, jt) else nc.vector
                eng.tensor_tensor(pt[:], q[:], mT_all[:, jt, :], op=Alu.mult)
                tile, off = pgt[h]
                for k in range(2):
                    nc.tensor.matmul(tile[off:off + 33, k * 512:(k + 1) * 512],
                                     Vpack[:, jt, h, :],
                                     pt[:, k * 512:(k + 1) * 512],
                                     start=(jt == 0), stop=(jt == NJT - 1))

            def fin(ftp, ndp, pgt, h):
                tile, off = pgt[h]
                numD = ndp.tile([33, IR], f16, tag="numD")
                nc.scalar.copy(numD[:], tile[off:off + 33, :])
                tpA = ftp.tile([128, 8, 33], f32, tag="tpA")
                for c in range(8):
                    nc.tensor.matmul(tpA[:, c, :],
                                     numD[:, c * 128:(c + 1) * 128],
                                     id16c[0:33, 0:33], start=True, stop=False)
                    nc.tensor.matmul(tpA[:, c, :],
                                     numD[32:33, c * 128:(c + 1) * 128],
                                     biasTE[32:33, h, :], start=False, stop=True)
                rdT = ndp.tile([128, 8, 1], f32, tag="rdT")
                nc.vector.reciprocal_approx_fast(rdT[:], tpA[:, :, 32:33])
                nc.vector.tensor_tensor(
                    out_sb[:, :, h * 32:(h + 1) * 32], tpA[:, :, 0:32],
                    rdT[:, :, 0:1].broadcast_to([128, 8, 32]), op=Alu.mult)
                nc.sync.dma_start(
                    out_d[:, h * 32:(h + 1) * 32].rearrange(
                        "(s p) f -> p s f", p=128),
                    out_sb[:, :, h * 32:(h + 1) * 32])

            def emit_body():
                psg_ctx = tc.tile_pool(name="psg", bufs=2, space="PSUM")
                psg = psg_ctx.__enter__()
                gqp_ctx = tc.tile_pool(name="gqp", bufs=10)
                gqp = gqp_ctx.__enter__()
                pgA = psg.tile([97, IR], f32, tag="pg", name="pgA")
                pgB = psg.tile([97, IR], f32, tag="pg", name="pgB")
                pgt = {0: (pgA, 0), 1: (pgA, 64), 2: (pgB, 0), 3: (pgB, 64)}

                # phase 1: masks stream; h0/h1 consume at production rate,
                # h2 lags two j-tiles
                with tc.tile_pool(name="adjp", bufs=2) as adjp, \
                     tc.tile_pool(name="mip", bufs=2) as mip, \
                     tc.tile_pool(name="mtp", bufs=2, space="PSUM") as mtp, \
                     tc.tile_pool(name="pvp", bufs=1, space="PSUM") as pvp:
                    for blk in range(8):
                        at = adjp.tile([128, 8, 256], f32, tag="adj")
                        nc.sync.dma_start(at[:], adj_r[:, :, blk * 256:(blk + 1) * 256])
                        mi = mip.tile([128, 8, 256], f16, tag="mi")
                        if blk in ACT_BIN:
                            nc.scalar.activation(
                                mi[:].rearrange("p a b -> p (a b)"),
                                at[:].rearrange("p a b -> p (a b)"),
                                Act.Sigmoid, bias=sigB[:, 0:1], scale=1e6)
                        else:
                            nc.vector.tensor_scalar(mi[:], at[:], 0.5, None,
                                                    op0=Alu.is_gt)
                        jt0 = 2 * blk
                        pv = pvp.tile([128, 512], f32, tag="pv", name="pv")
                        for d in range(2):
                            nc.tensor.matmul(
                                pv[:, d * 128:(d + 1) * 128],
                                xT16[:, (jt0 + d) * 128:(jt0 + d + 1) * 128],
                                Wf16[:], start=True, stop=True)
                        nc.vector.tensor_scalar(
                            Vpack[:, jt0:jt0 + 2, :, 0:32],
                            pv[:, 0:256].rearrange("p (j h f) -> p j h f", j=2, h=H),
                            0.0625, None, op0=Alu.mult)
                        for q in range(2):
                            jt = jt0 + q
                            mt = mtp.tile([128, IR], f16, tag="mt16", name="mt16")
                            for s in range(8):
                                nc.tensor.transpose(
                                    mt[:, s * 128:(s + 1) * 128],
                                    mi[:, s, q * 128:(q + 1) * 128], id16c[:])
                            nc.scalar.copy(mT_all[:, jt, :], mt[:])
                            pair_ops(gqp, pgt, 0, jt)
                            pair_ops(gqp, pgt, 1, jt)
                            if jt >= 2:
                                pair_ops(gqp, pgt, 2, jt - 2)

                # phase 2: h2 tail, h3, pipelined fins
                ftp_ctx = tc.tile_pool(name="ftp", bufs=2, space="PSUM")
                ftp = ftp_ctx.__enter__()
                ndp_ctx = tc.tile_pool(name="ndp", bufs=4)
                ndp = ndp_ctx.__enter__()
                try:
                    pair_ops(gqp, pgt, 2, NJT - 2)
                    pair_ops(gqp, pgt, 2, NJT - 1)
                    fin(ftp, ndp, pgt, 0)
                    for jt in range(NJT):
                        pair_ops(gqp, pgt, 3, jt)
                        if jt == 2:
                            fin(ftp, ndp, pgt, 1)
                        elif jt == 6:
                            fin(ftp, ndp, pgt, 2)
                    fin(ftp, ndp, pgt, 3)
                finally:
                    ndp_ctx.__exit__(None, None, None)
                    ftp_ctx.__exit__(None, None, None)
                    gqp_ctx.__exit__(None, None, None)
                    psg_ctx.__exit__(None, None, None)

            for _rep in range(reps):
                emit_body()
        finally:
            cst_ctx.__exit__(None, None, None)

    nc.compile()
    return nc


def _prepare_in_maps(x, adj, W, a_src, a_dst, bias):
    x = np.ascontiguousarray(np.asarray(x, dtype=np.float32))
    adj = np.asarray(adj, dtype=np.float32)
    W = np.asarray(W, dtype=np.float32)
    a_src = np.asarray(a_src, dtype=np.float32)
    a_dst = np.asarray(a_dst, dtype=np.float32)
    bias = np.asarray(bias, dtype=np.float32)

    Wf = np.ascontiguousarray(W.reshape(D, HF))
    aS = np.zeros((HF, H), np.float32)
    aD = np.zeros((HF, H), np.float32)
    for h in range(H):
        aS[h * F:(h + 1) * F, h] = a_src[h]
        aD[h * F:(h + 1) * F, h] = a_dst[h]
    biasRh = np.ascontiguousarray(bias.reshape(1, HF))

    in_maps = []
    for c in range(NCORES):
        b, cc = c // 2, c % 2
        i0 = cc * IR
        in_maps.append({
            "xT": np.ascontiguousarray(x[b].T),
            "xiT": np.ascontiguousarray(x[b, i0:i0 + IR].T),
            "adjS": np.ascontiguousarray(adj[b, i0:i0 + IR, :]),
            "Wf": Wf,
            "aS": aS,
            "aD": aD,
            "biasR": biasRh,
        })
    return in_maps


def run(inputs, trace=False, trace_cores=None):
    from concourse.bass_utils import run_bass_kernel_spmd
    if "nc" not in _CACHE:
        _CACHE["nc"] = build_nc()
    nc = _CACHE["nc"]
    in_maps = _prepare_in_maps(**inputs)
    kw = {}
    if trace:
        kw = dict(trace=True, trace_cores=trace_cores or [0])
    res = run_bass_kernel_spmd(nc, in_maps, list(range(NCORES)), **kw)
    out = np.zeros((B, N, HF), np.float32)
    for c in range(NCORES):
        b, cc = c // 2, c % 2
        out[b, cc * IR:(cc + 1) * IR, :] = res.results[c]["out"]
    return out, res


def kernel(**inputs):
    out, _ = run(inputs, trace=False)
    return out


# revision 20
# speedup vs baseline: 1.1646x; 1.0356x over previous
"""Batched GAT kernel for 8 Trainium2 NeuronCores.

Math: out[b,i,:] = softmax_j(mask(leakyrelu(s_i+t_j))) @ h  per head, concat heads.

Decomposition: exp(lrelu(e)) = max(u_i v_j, u'_i v'_j) with u=exp(s), v=exp(t),
u'=exp(.2s), v'=exp(.2t).  Dividing each row i by u'_i (cancels in softmax):
  p~_ij = m_ij * max(w_i v_j, v'_j),   w = exp(.8 s)
  out = (P~ @ [h|1]) -> num/den per head.  No G-indicator, no mask matmuls,
  no u-rescale combine: one matmul stream against a plain [h|1] fp16 pack.

Per core (c = 0..7): b = c//2, rows i in [ (c%2)*1024, +1024 ).
Per (head, j-tile): q = tensor_scalar(wb, *v_j, max v'_j) (DVE 4x) and
p~ = min(q, maskT) (DVE/GPSIMD 2x) where maskT in {0, BIG} comes from the
binarized adj transposed on PE via a regular matmul against BIG*I (scales the
mask for free).  Finalize per head: reciprocal_approx_fast on the den row,
fp16 ones-broadcast matmul, scale+bias, PE transpose out.
"""
import os
import sys
import numpy as np

for _p in ("/opt/trn_rl_repo",):
    if _p not in sys.path:
        sys.path.insert(0, _p)

B, N, D, H, F = 4, 2048, 128, 4, 32
HF = H * F           # 128
IR = 1024            # i-rows per core
NJT = N // 128       # 16 j-tiles
NCORES = 8

# engine assignment knobs
ACT_BIN = set(range(2, 8))         # binarize blocks on Act (sigmoid)
POOL_BIN = {0, 1}                  # binarize blocks on GPSIMD (is_gt)


def _on_pool(h, jt):
    # mask-mult engine choice: the lagged head (h2) runs on GPSIMD in phase 1
    # (its 2-jt lag absorbs the slower engine), h3 alternates in phase 2
    if h == 2:
        return jt < 14
    if h == 3:
        return jt % 2 == 1 and jt < 13
    return False

_CACHE = {}


def build_nc(reps=1):
    import concourse.bacc as bacc
    import concourse.tile as tile
    from concourse import mybir

    f32, f16 = mybir.dt.float32, mybir.dt.float16
    Alu = mybir.AluOpType
    Act = mybir.ActivationFunctionType

    nc = bacc.Bacc(None, target_bir_lowering=False)

    xT_d   = nc.dram_tensor("xT",   [D, N],    f32, kind="ExternalInput")
    xiT_d  = nc.dram_tensor("xiT",  [D, IR],   f32, kind="ExternalInput")
    adj_d  = nc.dram_tensor("adjS", [IR, N],   f32, kind="ExternalInput")
    Wf_d   = nc.dram_tensor("Wf",   [D, HF],   f32, kind="ExternalInput")
    aS_d   = nc.dram_tensor("aS",   [HF, H],   f32, kind="ExternalInput")
    aD_d   = nc.dram_tensor("aD",   [HF, H],   f32, kind="ExternalInput")
    bias_d = nc.dram_tensor("biasR", [1, HF],  f32, kind="ExternalInput")
    out_d  = nc.dram_tensor("out",  [IR, HF],  f32, kind="ExternalOutput")

    # host constants
    EY = np.zeros((4, 4 * 128), np.float16)
    for h in range(H):
        EY[h, h * 128:(h + 1) * 128] = 1.0
    EY_d = nc.inline_tensor(EY, "EYc")
    ID16_d = nc.inline_tensor(np.eye(128, dtype=np.float16), "id16c")


    adj_r = adj_d[:].rearrange("(s p) j -> p s j", p=128)

    with tile.TileContext(nc) as tc:
        cst_ctx = tc.tile_pool(name="cst", bufs=1)
        cst = cst_ctx.__enter__()
        try:
            xT   = cst.tile([D, N], f32)
            xiT  = cst.tile([D, IR], f32)
            Wf   = cst.tile([D, HF], f32)
            aS   = cst.tile([HF, H], f32)
            aD   = cst.tile([HF, H], f32)
            biasR = cst.tile([1, HF], f32)
            biasTE = cst.tile([64, 4, 33], f16)
            eyc  = cst.tile([4, 4 * 128], f16)
            id16c = cst.tile([128, 128], f16)
            sigB = cst.tile([128, 1], f32)

            Wf16 = cst.tile([D, HF], f16)
            aS16 = cst.tile([HF, H], f16)
            aD16 = cst.tile([HF, H], f16)
            xT16 = cst.tile([D, N], f16)
            xiT16 = cst.tile([D, IR], f16)
            hT16 = cst.tile([HF, N], f16)
            hiT16 = cst.tile([HF, IR], f16)
            warmA = cst.tile([1, 4], f32)
            tAll = cst.tile([128, NJT, H], f32)
            tv1  = cst.tile([128, NJT, H], f32)   # exp(t)
            tv2  = cst.tile([128, NJT, H], f32)   # exp(.2 t)
            sZ4  = cst.tile([4, IR], f32)
            w16  = cst.tile([4, IR], f16)         # exp(.8 s) fp16
            wb16 = cst.tile([128, H, IR], f16)    # broadcast of w16 per head
            Vpack = cst.tile([128, NJT, H, 33], f16)
            mT_all = cst.tile([128, NJT, IR], f16)
            out_sb = cst.tile([128, 8, HF], f32)

            nc.sync.dma_start(Wf[:], Wf_d[:])
            nc.sync.dma_start(xiT[:], xiT_d[:])
            nc.sync.dma_start(xT[:], xT_d[:])
            nc.sync.dma_start(aS[:], aS_d[:])
            nc.sync.dma_start(aD[:], aD_d[:])
            nc.sync.dma_start(eyc[:], EY_d[:])
            nc.sync.dma_start(id16c[:], ID16_d[:])
            nc.sync.dma_start(biasR[:], bias_d[:])
            nc.vector.memset(sigB[:], -5e5)
            nc.vector.memset(biasTE[:], 0.0)
            nc.scalar.copy(
                biasTE[32:33, :, 0:32],
                biasR[:].rearrange("p (h f) -> p h f", h=H))

            # ---------------- prep ----------------
            # warm every activation-table set during the DMA wait
            nc.scalar.copy(warmA[:, 0:1], sigB[0:1, 0:1])
            nc.scalar.activation(warmA[:, 1:2], sigB[0:1, 0:1], Act.Exp)
            nc.scalar.activation(warmA[:, 2:3], sigB[0:1, 0:1], Act.Sigmoid,
                                 bias=sigB[0:1, 0:1], scale=0.0)
            nc.scalar.copy(Wf16[:], Wf[:])
            nc.scalar.copy(aS16[:], aS[:])
            nc.scalar.copy(aD16[:], aD[:])
            nc.scalar.copy(xiT16[:], xiT[:])
            nc.scalar.copy(xT16[:], xT[:])
            with tc.tile_pool(name="pp", bufs=3, space="PSUM") as pp:
                # s chain: hiT -> sZ4 -> w16 -> wb16 (feeds the TS q-ops)
                for k in range(2):
                    ps = pp.tile([HF, 512], f32, tag="pp")
                    nc.tensor.matmul(ps[:], Wf16[:], xiT16[:, k * 512:(k + 1) * 512],
                                     start=True, stop=True)
                    nc.vector.tensor_copy(hiT16[:, k * 512:(k + 1) * 512], ps[:])
                for k in range(2):
                    ps = pp.tile([4, 512], f32, tag="pp")
                    nc.tensor.matmul(ps[:], aS16[:], hiT16[:, k * 512:(k + 1) * 512],
                                     start=True, stop=True)
                    nc.scalar.copy(sZ4[:, k * 512:(k + 1) * 512], ps[:])
                nc.scalar.activation(w16[:], sZ4[:], Act.Exp, scale=0.8)
                # t chain: hT -> tAll -> exps (feeds the TS scalars)
                for k in range(4):
                    ps = pp.tile([HF, 512], f32, tag="pp")
                    nc.tensor.matmul(ps[:], Wf16[:], xT16[:, k * 512:(k + 1) * 512],
                                     start=True, stop=True)
                    nc.vector.tensor_copy(hT16[:, k * 512:(k + 1) * 512], ps[:])
                for g in range(4):
                    ps = pp.tile([128, 4 * H], f32, tag="pp")
                    for k4 in range(4):
                        jt = g * 4 + k4
                        nc.tensor.matmul(ps[:, k4 * H:(k4 + 1) * H],
                                         hT16[:, jt * 128:(jt + 1) * 128], aD16[:],
                                         start=True, stop=True)
                    nc.scalar.copy(tAll[:, g * 4:(g + 1) * 4, :], ps[:])
                nc.scalar.activation(
                    tv1[:].rearrange("p a b -> p (a b)"),
                    tAll[:].rearrange("p a b -> p (a b)"), Act.Exp)
                nc.scalar.activation(
                    tv2[:].rearrange("p a b -> p (a b)"),
                    tAll[:].rearrange("p a b -> p (a b)"), Act.Exp, scale=0.2)
                # wb16: broadcast w16 rows to 128 partitions via PE
                for h in range(H):
                    for k in range(2):
                        ps = pp.tile([128, 512], f32, tag="pp")
                        nc.tensor.matmul(ps[:], eyc[:, h * 128:(h + 1) * 128],
                                         w16[:, k * 512:(k + 1) * 512],
                                         start=True, stop=True)
                        nc.vector.tensor_copy(wb16[:, h, k * 512:(k + 1) * 512], ps[:])
                # Vpack ones column (h/16 cols are drained inside the blk loop)
                nc.gpsimd.memset(Vpack[:, :, :, 32:33], 0.0625)

            # ---------------- main body (per rep) ----------------
            def pair_ops(gqp, pgt, h, jt):
                q = gqp.tile([128, IR], f16, tag="q")
                nc.vector.tensor_scalar(q[:], wb16[:, h, :],
                                        tv1[:, jt, h:h + 1],
                                        tv2[:, jt, h:h + 1],
                                        op0=Alu.mult, op1=Alu.max)
                pt = gqp.tile([128, IR], f16, tag="pt")
                eng = nc.gpsimd if _on_pool(h, jt) else nc.vector
                eng.tensor_tensor(pt[:], q[:], mT_all[:, jt, :], op=Alu.mult)
                tile, off = pgt[h]
                for k in range(2):
                    nc.tensor.matmul(tile[off:off + 33, k * 512:(k + 1) * 512],
                                     Vpack[:, jt, h, :],
                                     pt[:, k * 512:(k + 1) * 512],
                                     start=(jt == 0), stop=(jt == NJT - 1))

            def fin(ftp, ndp, pgt, h):
                tile, off = pgt[h]
                numD = ndp.tile([33, IR], f16, tag="numD")
                nc.scalar.copy(numD[:], tile[off:off + 33, :])
                tpA = ftp.tile([128, 8, 33], f32, tag="tpA")
                for c in range(8):
                    nc.tensor.matmul(tpA[:, c, :],
                                     numD[:, c * 128:(c + 1) * 128],
                                     id16c[0:33, 0:33], start=True, stop=False)
                    nc.tensor.matmul(tpA[:, c, :],
                                     numD[32:33, c * 128:(c + 1) * 128],
                                     biasTE[32:33, h, :], start=False, stop=True)
                rdT = ndp.tile([128, 8, 1], f32, tag="rdT")
                nc.vector.reciprocal_approx_fast(rdT[:], tpA[:, :, 32:33])
                nc.vector.tensor_tensor(
                    out_sb[:, :, h * 32:(h + 1) * 32], tpA[:, :, 0:32],
                    rdT[:, :, 0:1].broadcast_to([128, 8, 32]), op=Alu.mult)
                nc.sync.dma_start(
                    out_d[:, h * 32:(h + 1) * 32].rearrange(
                        "(s p) f -> p s f", p=128),
                    out_sb[:, :, h * 32:(h + 1) * 32])

            def emit_body():
                psg_ctx = tc.tile_pool(name="psg", bufs=2, space="PSUM")
                psg = psg_ctx.__enter__()
                gqp_ctx = tc.tile_pool(name="gqp", bufs=10)
                gqp = gqp_ctx.__enter__()
                pgA = psg.tile([97, IR], f32, tag="pg", name="pgA")
                pgB = psg.tile([97, IR], f32, tag="pg", name="pgB")
                pgt = {0: (pgA, 0), 1: (pgA, 64), 2: (pgB, 0), 3: (pgB, 64)}

                # phase 1: masks stream; h0/h1 consume at production rate,
                # h2 lags two j-tiles
                with tc.tile_pool(name="adjp", bufs=2) as adjp, \
                     tc.tile_pool(name="mip", bufs=2) as mip, \
                     tc.tile_pool(name="mtp", bufs=2, space="PSUM") as mtp, \
                     tc.tile_pool(name="pvp", bufs=1, space="PSUM") as pvp:
                    for blk in range(8):
                        at = adjp.tile([128, 8, 256], f32, tag="adj")
                        nc.sync.dma_start(at[:], adj_r[:, :, blk * 256:(blk + 1) * 256])
                        mi = mip.tile([128, 8, 256], f16, tag="mi")
                        if blk in ACT_BIN:
                            nc.scalar.activation(
                                mi[:].rearrange("p a b -> p (a b)"),
                                at[:].rearrange("p a b -> p (a b)"),
                                Act.Sigmoid, bias=sigB[:, 0:1], scale=1e6)
                        elif blk in POOL_BIN:
                            nc.gpsimd.tensor_scalar(mi[:], at[:], 0.5, None,
                                                    op0=Alu.is_gt)
                        else:
                            nc.vector.tensor_scalar(mi[:], at[:], 0.5, None,
                                                    op0=Alu.is_gt)
                        jt0 = 2 * blk
                        pv = pvp.tile([128, 512], f32, tag="pv", name="pv")
                        for d in range(2):
                            nc.tensor.matmul(
                                pv[:, d * 128:(d + 1) * 128],
                                xT16[:, (jt0 + d) * 128:(jt0 + d + 1) * 128],
                                Wf16[:], start=True, stop=True)
                        nc.vector.tensor_scalar(
                            Vpack[:, jt0:jt0 + 2, :, 0:32],
                            pv[:, 0:256].rearrange("p (j h f) -> p j h f", j=2, h=H),
                            0.0625, None, op0=Alu.mult)
                        for q in range(2):
                            jt = jt0 + q
                            mt = mtp.tile([128, IR], f16, tag="mt16", name="mt16")
                            for s in range(8):
                                nc.tensor.transpose(
                                    mt[:, s * 128:(s + 1) * 128],
                                    mi[:, s, q * 128:(q + 1) * 128], id16c[:])
                            nc.scalar.copy(mT_all[:, jt, :], mt[:])
                            pair_ops(gqp, pgt, 0, jt)
                            pair_ops(gqp, pgt, 1, jt)
                            if jt >= 2:
                                pair_ops(gqp, pgt, 2, jt - 2)

                # phase 2: h2 tail, h3, pipelined fins
                ftp_ctx = tc.tile_pool(name="ftp", bufs=2, space="PSUM")
                ftp = ftp_ctx.__enter__()
                ndp_ctx = tc.tile_pool(name="ndp", bufs=4)
                ndp = ndp_ctx.__enter__()
                try:
                    pair_ops(gqp, pgt, 2, NJT - 2)
                    pair_ops(gqp, pgt, 2, NJT - 1)
                    fin(ftp, ndp, pgt, 0)
                    for jt in range(NJT):
                        pair_ops(gqp, pgt, 3, jt)
                        if jt == 2:
                            fin(ftp, ndp, pgt, 1)
                        elif jt == 6:
                            fin(ftp, ndp, pgt, 2)
                    fin(ftp, ndp, pgt, 3)
                finally:
                    ndp_ctx.__exit__(None, None, None)
                    ftp_ctx.__exit__(None, None, None)
                    gqp_ctx.__exit__(None, None, None)
                    psg_ctx.__exit__(None, None, None)

            for _rep in range(reps):
                emit_body()
        finally:
            cst_ctx.__exit__(None, None, None)

    nc.compile()
    return nc


def _prepare_in_maps(x, adj, W, a_src, a_dst, bias):
    x = np.ascontiguousarray(np.asarray(x, dtype=np.float32))
    adj = np.asarray(adj, dtype=np.float32)
    W = np.asarray(W, dtype=np.float32)
    a_src = np.asarray(a_src, dtype=np.float32)
    a_dst = np.asarray(a_dst, dtype=np.float32)
    bias = np.asarray(bias, dtype=np.float32)

    Wf = np.ascontiguousarray(W.reshape(D, HF))
    aS = np.zeros((HF, H), np.float32)
    aD = np.zeros((HF, H), np.float32)
    for h in range(H):
        aS[h * F:(h + 1) * F, h] = a_src[h]
        aD[h * F:(h + 1) * F, h] = a_dst[h]
    biasRh = np.ascontiguousarray(bias.reshape(1, HF))

    in_maps = []
    for c in range(NCORES):
        b, cc = c // 2, c % 2
        i0 = cc * IR
        in_maps.append({
            "xT": np.ascontiguousarray(x[b].T),
            "xiT": np.ascontiguousarray(x[b, i0:i0 + IR].T),
            "adjS": np.ascontiguousarray(adj[b, i0:i0 + IR, :]),
            "Wf": Wf,
            "aS": aS,
            "aD": aD,
            "biasR": biasRh,
        })
    return in_maps


def run(inputs, trace=False, trace_cores=None):
    from concourse.bass_utils import run_bass_kernel_spmd
    if "nc" not in _CACHE:
        _CACHE["nc"] = build_nc()
    nc = _CACHE["nc"]
    in_maps = _prepare_in_maps(**inputs)
    kw = {}
    if trace:
        kw = dict(trace=True, trace_cores=trace_cores or [0])
    res = run_bass_kernel_spmd(nc, in_maps, list(range(NCORES)), **kw)
    out = np.zeros((B, N, HF), np.float32)
    for c in range(NCORES):
        b, cc = c // 2, c % 2
        out[b, cc * IR:(cc + 1) * IR, :] = res.results[c]["out"]
    return out, res


def kernel(**inputs):
    out, _ = run(inputs, trace=False)
    return out


# revision 21
# speedup vs baseline: 1.1750x; 1.0089x over previous
"""Batched GAT kernel for 8 Trainium2 NeuronCores.

Math: out[b,i,:] = softmax_j(mask(leakyrelu(s_i+t_j))) @ h  per head, concat heads.

Decomposition: exp(lrelu(e)) = max(u_i v_j, u'_i v'_j) with u=exp(s), v=exp(t),
u'=exp(.2s), v'=exp(.2t).  Dividing each row i by u'_i (cancels in softmax):
  p~_ij = m_ij * max(w_i v_j, v'_j),   w = exp(.8 s)
  out = (P~ @ [h|1]) -> num/den per head.  No G-indicator, no mask matmuls,
  no u-rescale combine: one matmul stream against a plain [h|1] fp16 pack.

Per core (c = 0..7): b = c//2, rows i in [ (c%2)*1024, +1024 ).
Per (head, j-tile): q = tensor_scalar(wb, *v_j, max v'_j) (DVE 4x) and
p~ = min(q, maskT) (DVE/GPSIMD 2x) where maskT in {0, BIG} comes from the
binarized adj transposed on PE via a regular matmul against BIG*I (scales the
mask for free).  Finalize per head: reciprocal_approx_fast on the den row,
fp16 ones-broadcast matmul, scale+bias, PE transpose out.
"""
import os
import sys
import numpy as np

for _p in ("/opt/trn_rl_repo",):
    if _p not in sys.path:
        sys.path.insert(0, _p)

B, N, D, H, F = 4, 2048, 128, 4, 32
HF = H * F           # 128
IR = 1024            # i-rows per core
NJT = N // 128       # 16 j-tiles
NCORES = 8

# engine assignment knobs
ACT_BIN = set(range(2, 8))         # binarize blocks on Act (sigmoid)
POOL_BIN = {0, 1}                  # binarize blocks on GPSIMD (is_gt)


def _on_pool(h, jt):
    # mask-mult engine choice: the lagged head (h2) runs on GPSIMD in phase 1
    # (its 2-jt lag absorbs the slower engine), h3 alternates in phase 2
    if h == 2:
        return jt < 15
    if h == 3:
        return jt % 2 == 1 and jt < 14
    return False

_CACHE = {}


def build_nc(reps=1):
    import concourse.bacc as bacc
    import concourse.tile as tile
    from concourse import mybir

    f32, f16 = mybir.dt.float32, mybir.dt.float16
    Alu = mybir.AluOpType
    Act = mybir.ActivationFunctionType

    nc = bacc.Bacc(None, target_bir_lowering=False)

    xT_d   = nc.dram_tensor("xT",   [D, N],    f32, kind="ExternalInput")
    xiT_d  = nc.dram_tensor("xiT",  [D, IR],   f32, kind="ExternalInput")
    adj_d  = nc.dram_tensor("adjS", [IR, N],   f32, kind="ExternalInput")
    Wf_d   = nc.dram_tensor("Wf",   [D, HF],   f32, kind="ExternalInput")
    aS_d   = nc.dram_tensor("aS",   [HF, H],   f32, kind="ExternalInput")
    aD_d   = nc.dram_tensor("aD",   [HF, H],   f32, kind="ExternalInput")
    bias_d = nc.dram_tensor("biasR", [1, HF],  f32, kind="ExternalInput")
    out_d  = nc.dram_tensor("out",  [IR, HF],  f32, kind="ExternalOutput")

    # host constants
    EY = np.zeros((4, 4 * 128), np.float16)
    for h in range(H):
        EY[h, h * 128:(h + 1) * 128] = 1.0
    EY_d = nc.inline_tensor(EY, "EYc")
    ID16_d = nc.inline_tensor(np.eye(128, dtype=np.float16), "id16c")


    adj_r = adj_d[:].rearrange("(s p) j -> p s j", p=128)

    with tile.TileContext(nc) as tc:
        cst_ctx = tc.tile_pool(name="cst", bufs=1)
        cst = cst_ctx.__enter__()
        try:
            xT   = cst.tile([D, N], f32)
            xiT  = cst.tile([D, IR], f32)
            Wf   = cst.tile([D, HF], f32)
            aS   = cst.tile([HF, H], f32)
            aD   = cst.tile([HF, H], f32)
            biasR = cst.tile([1, HF], f32)
            biasTE = cst.tile([64, 4, 33], f16)
            eyc  = cst.tile([4, 4 * 128], f16)
            id16c = cst.tile([128, 128], f16)
            sigB = cst.tile([128, 1], f32)

            Wf16 = cst.tile([D, HF], f16)
            aS16 = cst.tile([HF, H], f16)
            aD16 = cst.tile([HF, H], f16)
            xT16 = cst.tile([D, N], f16)
            xiT16 = cst.tile([D, IR], f16)
            hT16 = cst.tile([HF, N], f16)
            hiT16 = cst.tile([HF, IR], f16)
            warmA = cst.tile([1, 4], f32)
            tAll = cst.tile([128, NJT, H], f32)
            tv1  = cst.tile([128, NJT, H], f32)   # exp(t)
            tv2  = cst.tile([128, NJT, H], f32)   # exp(.2 t)
            sZ4  = cst.tile([4, IR], f32)
            w16  = cst.tile([4, IR], f16)         # exp(.8 s) fp16
            wb16 = cst.tile([128, H, IR], f16)    # broadcast of w16 per head
            Vpack = cst.tile([128, NJT, H, 33], f16)
            mT_all = cst.tile([128, NJT, IR], f16)
            out_sb = cst.tile([128, 8, HF], f32)

            nc.sync.dma_start(Wf[:], Wf_d[:])
            nc.sync.dma_start(xiT[:], xiT_d[:])
            nc.sync.dma_start(xT[:], xT_d[:])
            nc.sync.dma_start(aS[:], aS_d[:])
            nc.sync.dma_start(aD[:], aD_d[:])
            nc.sync.dma_start(eyc[:], EY_d[:])
            nc.sync.dma_start(id16c[:], ID16_d[:])
            nc.sync.dma_start(biasR[:], bias_d[:])
            nc.vector.memset(sigB[:], -5e5)
            nc.vector.memset(biasTE[:], 0.0)
            nc.scalar.copy(
                biasTE[32:33, :, 0:32],
                biasR[:].rearrange("p (h f) -> p h f", h=H))

            # ---------------- prep ----------------
            # warm every activation-table set during the DMA wait
            nc.scalar.copy(warmA[:, 0:1], sigB[0:1, 0:1])
            nc.scalar.activation(warmA[:, 1:2], sigB[0:1, 0:1], Act.Exp)
            nc.scalar.activation(warmA[:, 2:3], sigB[0:1, 0:1], Act.Sigmoid,
                                 bias=sigB[0:1, 0:1], scale=0.0)
            nc.scalar.copy(Wf16[:], Wf[:])
            nc.scalar.copy(aS16[:], aS[:])
            nc.scalar.copy(aD16[:], aD[:])
            nc.scalar.copy(xiT16[:], xiT[:])
            nc.scalar.copy(xT16[:], xT[:])
            with tc.tile_pool(name="pp", bufs=3, space="PSUM") as pp:
                # s chain: hiT -> sZ4 -> w16 -> wb16 (feeds the TS q-ops)
                for k in range(2):
                    ps = pp.tile([HF, 512], f32, tag="pp")
                    nc.tensor.matmul(ps[:], Wf16[:], xiT16[:, k * 512:(k + 1) * 512],
                                     start=True, stop=True)
                    nc.vector.tensor_copy(hiT16[:, k * 512:(k + 1) * 512], ps[:])
                for k in range(2):
                    ps = pp.tile([4, 512], f32, tag="pp")
                    nc.tensor.matmul(ps[:], aS16[:], hiT16[:, k * 512:(k + 1) * 512],
                                     start=True, stop=True)
                    nc.scalar.copy(sZ4[:, k * 512:(k + 1) * 512], ps[:])
                nc.scalar.activation(w16[:], sZ4[:], Act.Exp, scale=0.8)
                # t chain: hT -> tAll -> exps (feeds the TS scalars)
                for k in range(4):
                    ps = pp.tile([HF, 512], f32, tag="pp")
                    nc.tensor.matmul(ps[:], Wf16[:], xT16[:, k * 512:(k + 1) * 512],
                                     start=True, stop=True)
                    nc.vector.tensor_copy(hT16[:, k * 512:(k + 1) * 512], ps[:])
                for g in range(4):
                    ps = pp.tile([128, 4 * H], f32, tag="pp")
                    for k4 in range(4):
                        jt = g * 4 + k4
                        nc.tensor.matmul(ps[:, k4 * H:(k4 + 1) * H],
                                         hT16[:, jt * 128:(jt + 1) * 128], aD16[:],
                                         start=True, stop=True)
                    nc.scalar.copy(tAll[:, g * 4:(g + 1) * 4, :], ps[:])
                nc.scalar.activation(
                    tv1[:].rearrange("p a b -> p (a b)"),
                    tAll[:].rearrange("p a b -> p (a b)"), Act.Exp)
                nc.scalar.activation(
                    tv2[:].rearrange("p a b -> p (a b)"),
                    tAll[:].rearrange("p a b -> p (a b)"), Act.Exp, scale=0.2)
                # wb16: broadcast w16 rows to 128 partitions via PE
                for h in range(H):
                    for k in range(2):
                        ps = pp.tile([128, 512], f32, tag="pp")
                        nc.tensor.matmul(ps[:], eyc[:, h * 128:(h + 1) * 128],
                                         w16[:, k * 512:(k + 1) * 512],
                                         start=True, stop=True)
                        nc.vector.tensor_copy(wb16[:, h, k * 512:(k + 1) * 512], ps[:])
                # Vpack ones column (h/16 cols are drained inside the blk loop)
                nc.gpsimd.memset(Vpack[:, :, :, 32:33], 0.0625)

            # ---------------- main body (per rep) ----------------
            def pair_ops(gqp, pgt, h, jt):
                q = gqp.tile([128, IR], f16, tag="q")
                nc.vector.tensor_scalar(q[:], wb16[:, h, :],
                                        tv1[:, jt, h:h + 1],
                                        tv2[:, jt, h:h + 1],
                                        op0=Alu.mult, op1=Alu.max)
                pt = gqp.tile([128, IR], f16, tag="pt")
                eng = nc.gpsimd if _on_pool(h, jt) else nc.vector
                eng.tensor_tensor(pt[:], q[:], mT_all[:, jt, :], op=Alu.mult)
                tile, off = pgt[h]
                for k in range(2):
                    nc.tensor.matmul(tile[off:off + 33, k * 512:(k + 1) * 512],
                                     Vpack[:, jt, h, :],
                                     pt[:, k * 512:(k + 1) * 512],
                                     start=(jt == 0), stop=(jt == NJT - 1))

            def fin(ftp, ndp, pgt, h):
                tile, off = pgt[h]
                numD = ndp.tile([33, IR], f16, tag="numD")
                nc.scalar.copy(numD[:], tile[off:off + 33, :])
                tpA = ftp.tile([128, 8, 33], f32, tag="tpA")
                for c in range(8):
                    nc.tensor.matmul(tpA[:, c, :],
                                     numD[:, c * 128:(c + 1) * 128],
                                     id16c[0:33, 0:33], start=True, stop=False)
                    nc.tensor.matmul(tpA[:, c, :],
                                     numD[32:33, c * 128:(c + 1) * 128],
                                     biasTE[32:33, h, :], start=False, stop=True)
                rdT = ndp.tile([128, 8, 1], f32, tag="rdT")
                nc.vector.reciprocal_approx_fast(rdT[:], tpA[:, :, 32:33])
                nc.vector.tensor_tensor(
                    out_sb[:, :, h * 32:(h + 1) * 32], tpA[:, :, 0:32],
                    rdT[:, :, 0:1].broadcast_to([128, 8, 32]), op=Alu.mult)
                nc.sync.dma_start(
                    out_d[:, h * 32:(h + 1) * 32].rearrange(
                        "(s p) f -> p s f", p=128),
                    out_sb[:, :, h * 32:(h + 1) * 32])

            def emit_body():
                psg_ctx = tc.tile_pool(name="psg", bufs=2, space="PSUM")
                psg = psg_ctx.__enter__()
                gqp_ctx = tc.tile_pool(name="gqp", bufs=14)
                gqp = gqp_ctx.__enter__()
                pgA = psg.tile([97, IR], f32, tag="pg", name="pgA")
                pgB = psg.tile([97, IR], f32, tag="pg", name="pgB")
                pgt = {0: (pgA, 0), 1: (pgA, 64), 2: (pgB, 0), 3: (pgB, 64)}

                # phase 1: masks stream; h0/h1 consume at production rate,
                # h2 lags two j-tiles
                with tc.tile_pool(name="adjp", bufs=2) as adjp, \
                     tc.tile_pool(name="mip", bufs=2) as mip, \
                     tc.tile_pool(name="mtp", bufs=2, space="PSUM") as mtp, \
                     tc.tile_pool(name="pvp", bufs=1, space="PSUM") as pvp:
                    for blk in range(8):
                        at = adjp.tile([128, 8, 256], f32, tag="adj")
                        nc.sync.dma_start(at[:], adj_r[:, :, blk * 256:(blk + 1) * 256])
                        mi = mip.tile([128, 8, 256], f16, tag="mi")
                        if blk in ACT_BIN:
                            nc.scalar.activation(
                                mi[:].rearrange("p a b -> p (a b)"),
                                at[:].rearrange("p a b -> p (a b)"),
                                Act.Sigmoid, bias=sigB[:, 0:1], scale=1e6)
                        elif blk in POOL_BIN:
                            nc.gpsimd.tensor_scalar(mi[:], at[:], 0.5, None,
                                                    op0=Alu.is_gt)
                        else:
                            nc.vector.tensor_scalar(mi[:], at[:], 0.5, None,
                                                    op0=Alu.is_gt)
                        jt0 = 2 * blk
                        pv = pvp.tile([128, 512], f32, tag="pv", name="pv")
                        for d in range(2):
                            nc.tensor.matmul(
                                pv[:, d * 128:(d + 1) * 128],
                                xT16[:, (jt0 + d) * 128:(jt0 + d + 1) * 128],
                                Wf16[:], start=True, stop=True)
                        if blk < 4:
                            nc.vector.tensor_scalar(
                                Vpack[:, jt0:jt0 + 2, :, 0:32],
                                pv[:, 0:256].rearrange("p (j h f) -> p j h f",
                                                       j=2, h=H),
                                0.0625, None, op0=Alu.mult)
                        else:
                            nc.scalar.mul(
                                Vpack[:, jt0:jt0 + 2, :, 0:32],
                                pv[:, 0:256].rearrange("p (j h f) -> p j h f",
                                                       j=2, h=H), 0.0625)
                        for q in range(2):
                            jt = jt0 + q
                            mt = mtp.tile([128, IR], f16, tag="mt16", name="mt16")
                            for s in range(8):
                                nc.tensor.transpose(
                                    mt[:, s * 128:(s + 1) * 128],
                                    mi[:, s, q * 128:(q + 1) * 128], id16c[:])
                            nc.scalar.copy(mT_all[:, jt, :], mt[:])
                            pair_ops(gqp, pgt, 0, jt)
                            pair_ops(gqp, pgt, 1, jt)
                            if jt >= 1:
                                pair_ops(gqp, pgt, 2, jt - 1)

                # phase 2: h2 tail, h3, pipelined fins
                ftp_ctx = tc.tile_pool(name="ftp", bufs=2, space="PSUM")
                ftp = ftp_ctx.__enter__()
                ndp_ctx = tc.tile_pool(name="ndp", bufs=4)
                ndp = ndp_ctx.__enter__()
                try:
                    pair_ops(gqp, pgt, 2, NJT - 1)
                    fin(ftp, ndp, pgt, 0)
                    for jt in range(NJT):
                        pair_ops(gqp, pgt, 3, jt)
                        if jt == 2:
                            fin(ftp, ndp, pgt, 1)
                        elif jt == 6:
                            fin(ftp, ndp, pgt, 2)
                    fin(ftp, ndp, pgt, 3)
                finally:
                    ndp_ctx.__exit__(None, None, None)
                    ftp_ctx.__exit__(None, None, None)
                    gqp_ctx.__exit__(None, None, None)
                    psg_ctx.__exit__(None, None, None)

            for _rep in range(reps):
                emit_body()
        finally:
            cst_ctx.__exit__(None, None, None)

    nc.compile()
    return nc


def _prepare_in_maps(x, adj, W, a_src, a_dst, bias):
    x = np.ascontiguousarray(np.asarray(x, dtype=np.float32))
    adj = np.asarray(adj, dtype=np.float32)
    W = np.asarray(W, dtype=np.float32)
    a_src = np.asarray(a_src, dtype=np.float32)
    a_dst = np.asarray(a_dst, dtype=np.float32)
    bias = np.asarray(bias, dtype=np.float32)

    Wf = np.ascontiguousarray(W.reshape(D, HF))
    aS = np.zeros((HF, H), np.float32)
    aD = np.zeros((HF, H), np.float32)
    for h in range(H):
        aS[h * F:(h + 1) * F, h] = a_src[h]
        aD[h * F:(h + 1) * F, h] = a_dst[h]
    biasRh = np.ascontiguousarray(bias.reshape(1, HF))

    in_maps = []
    for c in range(NCORES):
        b, cc = c // 2, c % 2
        i0 = cc * IR
        in_maps.append({
            "xT": np.ascontiguousarray(x[b].T),
            "xiT": np.ascontiguousarray(x[b, i0:i0 + IR].T),
            "adjS": np.ascontiguousarray(adj[b, i0:i0 + IR, :]),
            "Wf": Wf,
            "aS": aS,
            "aD": aD,
            "biasR": biasRh,
        })
    return in_maps


def run(inputs, trace=False, trace_cores=None):
    from concourse.bass_utils import run_bass_kernel_spmd
    if "nc" not in _CACHE:
        _CACHE["nc"] = build_nc()
    nc = _CACHE["nc"]
    in_maps = _prepare_in_maps(**inputs)
    kw = {}
    if trace:
        kw = dict(trace=True, trace_cores=trace_cores or [0])
    res = run_bass_kernel_spmd(nc, in_maps, list(range(NCORES)), **kw)
    out = np.zeros((B, N, HF), np.float32)
    for c in range(NCORES):
        b, cc = c // 2, c % 2
        out[b, cc * IR:(cc + 1) * IR, :] = res.results[c]["out"]
    return out, res


def kernel(**inputs):
    out, _ = run(inputs, trace=False)
    return out
